# revision 7
# baseline (speedup 1.0000x reference)
"""Causal attention kernel for Trainium2, 8 NeuronCores — K/V pair-split.

Like kernel.py (8 cores = 4 batches x 2 interleaved query-sets), but the K
projection is no longer duplicated within a batch pair: each core projects
K^T only for its OWN tokens (every other 256-chunk; host feeds x^T of them),
and the pair exchanges halves with two chunked 2-rank AllGathers that finish
well before attention needs K. V stays duplicated: the full V+K exchange
does not fit the collective-latency window (measured ~35us lead-in + ~18us
per 1MB AllGather), and a late V stalls the PE far longer than the 27us of
duplicated V compute.

Per-core PE work drops from 484K to 419K cycles. Numerics are identical to
kernel.py (bf16 matmuls, fp32 PSUM/softmax, no max-subtraction).
"""

import numpy as np
import ml_dtypes
from contextlib import ExitStack

import concourse.bacc as bacc
import concourse.tile as tile
from concourse.tile import add_dep_helper
from concourse import mybir
from concourse.bass_utils import run_bass_kernel_spmd
from concourse.masks import make_identity

B = 4          # batch
S = 2048       # sequence length
D = 1024       # d_in = d_out
NCORES = 8
QB = 128       # query block rows
NQB = S // QB // 2   # 8 q-blocks per core
SQ = NQB * QB        # 1024 query rows per core
SH = S // 2          # per-core projection half
KC = 256             # causal key-extent granularity
SC = 512             # score-chunk width (psum bank)
NDC = D // 128       # 8 contraction chunks
SCALE = 1.0 / float(np.sqrt(D))
MASK_VAL = -1e10
GROUPS = [[0, 1], [2, 3], [4, 5], [6, 7]]

BF = mybir.dt.bfloat16
F32 = mybir.dt.float32


def build_program():
    nc = bacc.Bacc("TRN2", target_bir_lowering=False, debug=False,
                   num_devices=NCORES)

    # host-packed inputs (all bf16 except the mask):
    #   xth: x^T of the core's own token-half, [128, shh, dc, 512]
    #   xqt: x^T of the core's own queries, sh-major [128, sh, dc, 512]
    #   wk/wq: ec-major [128, ec, dc, 128]; wv: eh-major [128, eh, dc, 512]
    # token-major quarters so the V projection can start after 1MB arrives
    xt_d = nc.dram_tensor("xt", [128, 4, NDC, SC], BF, kind="ExternalInput")
    xth_d = nc.dram_tensor("xth", [128, 2, NDC, SC], BF, kind="ExternalInput")
    xqt_d = nc.dram_tensor("xqt", [128, SQ // SC, NDC, SC], BF,
                           kind="ExternalInput")
    wq_d = nc.dram_tensor("wq", [128, NDC, NDC, 128], BF, kind="ExternalInput")
    wk_d = nc.dram_tensor("wk", [128, NDC, NDC, 128], BF, kind="ExternalInput")
    wv_d = nc.dram_tensor("wv", [128, NDC, D], BF, kind="ExternalInput")
    msk_d = nc.dram_tensor("msk", [QB, KC], F32, kind="ExternalInput")
    out_d = nc.dram_tensor("out", [NQB, QB, D], F32, kind="ExternalOutput")

    # collective bounce buffers (internal DRAM; collectives can't touch I/O)
    kin = [nc.dram_tensor(f"kin{s}", [128, NDC, SC], BF) for s in range(2)]
    kout = [nc.dram_tensor(f"kout{s}", [2, 128, NDC, SC], BF) for s in range(2)]

    with tile.TileContext(nc) as tc, ExitStack() as ctx:
        consts = ctx.enter_context(tc.tile_pool(name="consts", bufs=1))
        persist = ctx.enter_context(tc.tile_pool(name="persist", bufs=1))

        ident = consts.tile([128, 128], BF, name="ident")
        make_identity(nc, ident)
        msk_sb = consts.tile([QB, KC], F32, name="msk_sb")
        nc.sync.dma_start(out=msk_sb, in_=msk_d.ap())

        # Persistent activations (partition = head dim for QT/KT, = keys for V)
        QT = persist.tile([128, NDC, SQ], BF, name="QT")   # Q^T, pre-scaled
        KT = persist.tile([128, NDC, S], BF, name="KT")    # K^T (gathered)
        V = persist.tile([128, S // 128, D], BF, name="V")  # V rows (gathered)

        # ---------------- projection phase ----------------
        with tc.tile_pool(name="proj_sb", bufs=1) as proj_sb, \
             tc.tile_pool(name="proj_ps", bufs=4, space="PSUM") as proj_ps:
            # PE warmup on the on-chip identity (no DMA dep): ramps the PE
            # p-state during the framework prologue. Result discarded.
            warm_ps = proj_ps.tile([128, SC], F32, name="warm_ps", tag="warm",
                                   bufs=1)
            for _ in range(10):
                nc.tensor.matmul(
                    warm_ps[:, 0:128], lhsT=ident, rhs=ident, start=True,
                    stop=True,
                )

            xT = proj_sb.tile([128, 4, NDC, SC], BF, name="xT")
            xth = proj_sb.tile([128, 2, NDC, SC], BF, name="xth")
            xqT = proj_sb.tile([128, SQ // SC, NDC, SC], BF, name="xqT")
            wq_sb = proj_sb.tile([128, NDC, NDC, 128], BF, name="wq_sb")
            wk_sb = proj_sb.tile([128, NDC, NDC, 128], BF, name="wk_sb")
            wv_sb = proj_sb.tile([128, NDC, D], BF, name="wv_sb")
            # single 512-col staging buffer, reused by both K stages (the
            # stage-0 DMA to kin drains well before stage 1's copies land)
            KH = proj_sb.tile([128, NDC, SC], BF, name="KH")

            # DMA priority: the K feed (sync engine) and the V feed
            # (gpsimd engine) stream in parallel from t=0 — the K projection
            # is PE-bound shortly after its first chunks land, while the V
            # projection otherwise stalls ~6us waiting on wv/xT. wv arrives
            # in eh-halves so V's first group needs 2MB, not 3MB. The Q feed
            # is chained last (needed last).
            nc.sync.dma_start(out=wk_sb[:, 0], in_=wk_d.ap()[:, 0])
            nc.sync.dma_start(out=xth[:, 0], in_=xth_d.ap()[:, 0])
            for ec in range(1, NDC):
                nc.sync.dma_start(out=wk_sb[:, ec], in_=wk_d.ap()[:, ec])
            nc.sync.dma_start(out=xth[:, 1], in_=xth_d.ap()[:, 1])
            nc.gpsimd.dma_start(out=wv_sb[:, :, 0:SC], in_=wv_d.ap()[:, :, 0:SC])
            prev = nc.gpsimd.dma_start(out=xT[:, 0], in_=xt_d.ap()[:, 0])
            nc.gpsimd.dma_start(out=wv_sb[:, :, SC:D], in_=wv_d.ap()[:, :, SC:D])
            for qu in range(1, 4):
                prev = nc.gpsimd.dma_start(
                    out=xT[:, qu], in_=xt_d.ap()[:, qu]
                )
            i = nc.gpsimd.dma_start(out=wq_sb, in_=wq_d.ap())
            add_dep_helper(i.ins, prev.ins, reason="dma phase order")
            for sh in range(SQ // SC):
                i = nc.gpsimd.dma_start(out=xqT[:, sh], in_=xqt_d.ap()[:, sh])
                add_dep_helper(i.ins, prev.ins, reason="dma phase order")

            def k_half(sh):
                # K^T[e, own-half tokens sh*512..] = sum_d Wk[d,e] xh^T[d, s]
                for ec in range(NDC):
                    pp = proj_ps.tile([128, SC], F32, name="pp")
                    for dc in range(NDC):
                        nc.tensor.matmul(
                            pp,
                            lhsT=wk_sb[:, ec, dc, :],
                            rhs=xth[:, sh, dc, :],
                            start=(dc == 0),
                            stop=(dc == NDC - 1),
                        )
                    nc.scalar.copy(KH[:, ec, :], pp)
                # stage + exchange. Tokens are interleaved between the pair by
                # 256-chunks (rank r owns global chunks {2*lc + r}), so this
                # stage's AllGather delivers a CONTIGUOUS early key range
                # [s*1024, (s+1)*1024) — matching the causal need order.
                d1 = nc.sync.dma_start(out=kin[sh].ap(), in_=KH)
                cc = nc.gpsimd.collective_compute(
                    "AllGather", mybir.AluOpType.bypass,
                    replica_groups=GROUPS,
                    ins=[kin[sh].ap().opt()],
                    outs=[kout[sh].ap().opt()],
                )
                add_dep_helper(cc.ins, d1.ins, reason="cc after staging dma")
                for r in range(2):
                    for m in range(2):
                        g = 2 * (2 * sh + m) + r   # global 256-chunk index
                        d2 = nc.sync.dma_start(
                            out=KT[:, :, g * KC:(g + 1) * KC],
                            in_=kout[sh].ap()[r][:, :, m * KC:(m + 1) * KC],
                        )
                        add_dep_helper(d2.ins, cc.ins, reason="read gathered")

            # K halves + exchanges first (max collective lead time),
            # then the full (duplicated) V projection, then Q
            k_half(0)
            k_half(1)

            # V[s, e] = sum_d x^T[d, s] * Wv[d, e]
            for kb in range(S // 128):
                for eh in range(D // SC):
                    pp = proj_ps.tile([128, SC], F32, name="pp")
                    for dc in range(NDC):
                        nc.tensor.matmul(
                            pp,
                            lhsT=xT[:, kb // 4, dc,
                                    (kb % 4) * 128:(kb % 4 + 1) * 128],
                            rhs=wv_sb[:, dc, eh * SC:(eh + 1) * SC],
                            start=(dc == 0),
                            stop=(dc == NDC - 1),
                        )
                    nc.scalar.copy(V[:, kb, eh * SC:(eh + 1) * SC], pp)

            # Q^T[e, s] = sum_d Wq[d, e] * xq^T[d, s]   (scale folded in)
            for sh in range(SQ // SC):
                for ec in range(NDC):
                    pp = proj_ps.tile([128, SC], F32, name="pp")
                    for dc in range(NDC):
                        nc.tensor.matmul(
                            pp,
                            lhsT=wq_sb[:, ec, dc, :],
                            rhs=xqT[:, sh, dc, :],
                            start=(dc == 0),
                            stop=(dc == NDC - 1),
                        )
                    nc.scalar.mul(QT[:, ec, sh * SC:(sh + 1) * SC], pp, SCALE)

        # ---------------- attention phase ----------------
        with tc.tile_pool(name="att_sb", bufs=2) as att_sb, \
             tc.tile_pool(name="pt_sb_pool", bufs=20) as pt_pool, \
             tc.tile_pool(name="stat_sb", bufs=4) as stat_sb, \
             tc.tile_pool(name="att_ps", bufs=1, space="PSUM") as att_ps:
            for j in reversed(range(NQB)):
                ext = (j + 1) * KC            # causal key extent for block j
                nch = (ext + SC - 1) // SC    # score chunks (512 wide, last may be 256)
                qsl = slice(j * 128, (j + 1) * 128)

                # Scores are ~N(0,1) after the folded 1/sqrt(D) scaling, so
                # exp() without max-subtraction is numerically safe; dropping
                # the global row-max removes the per-block barrier and lets
                # scores -> exp -> transpose -> AV pipeline per 512-chunk.
                P = att_sb.tile([128, NQB * KC], BF, name="P", tag="P")
                rsum = stat_sb.tile([128, nch], F32, name="rsum", tag="rsum")
                pts = []

                # software pipeline: scores(c+1) is issued on the PE before
                # the transposes of chunk c, so the PE crunches the next
                # chunk's scores while the scalar engine runs exp(c).
                def do_scores(c):
                    w = min(SC, ext - c * SC)
                    ps_c = att_ps.tile([128, SC], F32, name="ps_sc",
                                       tag="ps_sc", bufs=4)
                    for ec in range(NDC):
                        nc.tensor.matmul(
                            ps_c[:, 0:w],
                            lhsT=QT[:, ec, qsl],
                            rhs=KT[:, ec, c * SC:c * SC + w],
                            start=(ec == 0),
                            stop=(ec == NDC - 1),
                        )
                    if c == nch - 1:
                        # causal mask on the last KC columns
                        nc.vector.tensor_add(
                            out=ps_c[:, w - KC:w],
                            in0=ps_c[:, w - KC:w],
                            in1=msk_sb,
                        )
                    return ps_c, w

                def do_exp_transpose(c, ps_c, w):
                    nc.scalar.activation(
                        P[:, c * SC:c * SC + w],
                        ps_c[:, 0:w],
                        mybir.ActivationFunctionType.Exp,
                        bias=0.0,
                        scale=1.0,
                        accum_out=rsum[:, c:c + 1],
                    )
                    for kb in range(w // 128):
                        pt_ps = att_ps.tile(
                            [128, 128], BF, name="pt_ps", tag="pt_ps", bufs=2
                        )
                        nc.tensor.transpose(
                            pt_ps,
                            P[:, c * SC + kb * 128:c * SC + (kb + 1) * 128],
                            ident,
                        )
                        pt = pt_pool.tile([128, 128], BF, name="pt", tag="pt")
                        nc.vector.tensor_copy(pt, pt_ps)
                        pts.append(pt)

                pend = do_scores(0)
                for c in range(nch):
                    nxt = do_scores(c + 1) if c + 1 < nch else None
                    do_exp_transpose(c, *pend)
                    pend = nxt

                tsum = stat_sb.tile([128, 1], F32, name="tsum", tag="tsum")
                nc.vector.reduce_sum(tsum, rsum, axis=mybir.AxisListType.X)
                rinv = stat_sb.tile([128, 1], F32, name="rinv", tag="rinv")
                nc.vector.reciprocal(rinv, tsum)

                # out[q, e] = sum_k P^T[k, q]^T V[k, e]
                ps_o = []
                for eh in range(D // SC):
                    # the final block borrows the bank-padded transpose-staging
                    # slots so it never waits on the previous block's normalize
                    avtag = "pt_ps" if j == 0 else "ps_av"
                    ps_av = att_ps.tile([128, SC], F32, name="ps_av",
                                        tag=avtag, bufs=2)
                    for kb in range(ext // 128):
                        nc.tensor.matmul(
                            ps_av,
                            lhsT=pts[kb],
                            rhs=V[:, kb, eh * SC:(eh + 1) * SC],
                            start=(kb == 0),
                            stop=(kb == ext // 128 - 1),
                        )
                    ps_o.append(ps_av)

                # normalize + store per half so the first DMA overlaps the
                # second normalize (matters for the last block's tail)
                ob = att_sb.tile([128, D], F32, name="ob", tag="ob")
                for eh in range(D // SC):
                    nc.scalar.mul(ob[:, eh * SC:(eh + 1) * SC], ps_o[eh], rinv)
                    nc.sync.dma_start(
                        out=out_d.ap()[j][:, eh * SC:(eh + 1) * SC],
                        in_=ob[:, eh * SC:(eh + 1) * SC],
                    )

    nc.compile()
    return nc


_PROGRAM = None


def _get_program():
    global _PROGRAM
    if _PROGRAM is None:
        _PROGRAM = build_program()
    return _PROGRAM


def _pack_w(w):
    # [D, D] -> [128, NDC, D]: partition p, chunk dc holds row dc*128+p
    bf = ml_dtypes.bfloat16
    return np.ascontiguousarray(
        w.astype(bf).reshape(NDC, 128, D).transpose(1, 0, 2)
    )


def _pack_w_ec(w):
    # ec-major: [128, ec, dc, 128] so each ec-chunk is one small priority DMA
    return np.ascontiguousarray(
        _pack_w(w).reshape(128, NDC, NDC, 128).transpose(0, 2, 1, 3)
    )


def _pack_xt(xr):
    # [rows, D] -> x^T packed [128, NDC, rows]
    bf = ml_dtypes.bfloat16
    return np.ascontiguousarray(
        xr.astype(bf).T.reshape(NDC, 128, xr.shape[0]).transpose(1, 0, 2)
    )


def make_in_maps(x, Wq, Wk, Wv):
    wqb = _pack_w_ec(Wq)
    wkb = _pack_w_ec(Wk)
    wvb = _pack_w(Wv)
    r = np.arange(QB)[:, None]
    cc = np.arange(KC)[None, :]
    in_maps = []
    for c in range(NCORES):
        b, t = c // 2, c % 2
        xb = x[b]
        xqb = xb.reshape(S // QB, QB, D)[t::2].reshape(SQ, D)
        xqtb = np.ascontiguousarray(
            _pack_xt(xqb).reshape(128, NDC, SQ // SC, SC).transpose(0, 2, 1, 3)
        )
        # own tokens = every other 256-chunk (rank r owns global chunks
        # {2*lc + r}), packed [128, shh, dc, 512]
        xh_rows = xb.reshape(S // KC, KC, D)[t::2].reshape(SH, D)
        xhb = np.ascontiguousarray(
            _pack_xt(xh_rows)
            .reshape(128, NDC, 2, SC).transpose(0, 2, 1, 3)
        )
        mask = np.where(cc <= t * QB + r, 0.0, MASK_VAL).astype(np.float32)
        xtb = np.ascontiguousarray(
            _pack_xt(xb).reshape(128, NDC, 4, SC).transpose(0, 2, 1, 3)
        )
        in_maps.append(
            {"xt": xtb, "xth": xhb, "xqt": xqtb,
             "wq": wqb, "wk": wkb, "wv": wvb, "msk": mask}
        )
    return in_maps


def assemble_output(results):
    out = np.empty((B, S, D), dtype=np.float32)
    ov = out.reshape(B, S // QB, QB, D)
    for c in range(NCORES):
        b, t = c // 2, c % 2
        ov[b, t::2] = results[c]["out"]
    return out


def kernel(x, Wq, Wk, Wv):
    x = np.asarray(x)
    nc = _get_program()
    in_maps = make_in_maps(x, np.asarray(Wq), np.asarray(Wk), np.asarray(Wv))
    res = run_bass_kernel_spmd(nc, in_maps, list(range(NCORES))).results
    return assemble_output(res)


# revision 8
# speedup vs baseline: 1.0822x; 1.0822x over previous
"""Causal attention kernel for Trainium2, 8 NeuronCores — K/V pair-split.

Like kernel.py (8 cores = 4 batches x 2 interleaved query-sets), but the K
projection is no longer duplicated within a batch pair: each core projects
K^T only for its OWN tokens (every other 256-chunk; host feeds x^T of them),
and the pair exchanges halves with two chunked 2-rank AllGathers that finish
well before attention needs K. V stays duplicated: the full V+K exchange
does not fit the collective-latency window (measured ~35us lead-in + ~18us
per 1MB AllGather), and a late V stalls the PE far longer than the 27us of
duplicated V compute.

Per-core PE work drops from 484K to 419K cycles. Numerics are identical to
kernel.py (bf16 matmuls, fp32 PSUM/softmax, no max-subtraction).
"""

import numpy as np
import ml_dtypes
from contextlib import ExitStack

import concourse.bacc as bacc
import concourse.tile as tile
from concourse.tile import add_dep_helper
from concourse import mybir
from concourse.bass_utils import run_bass_kernel_spmd
from concourse.masks import make_identity

B = 4          # batch
S = 2048       # sequence length
D = 1024       # d_in = d_out
NCORES = 8
QB = 128       # query block rows
NQB = S // QB // 2   # 8 q-blocks per core
SQ = NQB * QB        # 1024 query rows per core
SH = S // 2          # per-core projection half
KC = 256             # causal key-extent granularity
SC = 512             # score-chunk width (psum bank)
NDC = D // 128       # 8 contraction chunks
SCALE = 1.0 / float(np.sqrt(D))
MASK_VAL = -1e10
GROUPS = [[0, 1], [2, 3], [4, 5], [6, 7]]

BF = mybir.dt.bfloat16
F32 = mybir.dt.float32


def build_program():
    nc = bacc.Bacc("TRN2", target_bir_lowering=False, debug=False,
                   num_devices=NCORES)

    # host-packed inputs (all bf16 except the mask):
    #   xth: x^T of the core's own token-half, [128, shh, dc, 512]
    #   xqt: x^T of the core's own queries, sh-major [128, sh, dc, 512]
    #   wk/wq: ec-major [128, ec, dc, 128]; wv: eh-major [128, eh, dc, 512]
    # token-major quarters so the V projection can start after 1MB arrives
    xt_d = nc.dram_tensor("xt", [128, 4, NDC, SC], BF, kind="ExternalInput")
    xth_d = nc.dram_tensor("xth", [128, 2, NDC, SC], BF, kind="ExternalInput")
    xqt_d = nc.dram_tensor("xqt", [128, SQ // SC, NDC, SC], BF,
                           kind="ExternalInput")
    wq_d = nc.dram_tensor("wq", [128, NDC, NDC, 128], BF, kind="ExternalInput")
    wk_d = nc.dram_tensor("wk", [128, NDC, NDC, 128], BF, kind="ExternalInput")
    wv_d = nc.dram_tensor("wv", [128, NDC, D], BF, kind="ExternalInput")
    msk_d = nc.dram_tensor("msk", [QB, KC], F32, kind="ExternalInput")
    out_d = nc.dram_tensor("out", [NQB, QB, D], F32, kind="ExternalOutput")

    # collective bounce buffers (internal DRAM; collectives can't touch I/O)
    kin = [nc.dram_tensor(f"kin{s}", [128, NDC, SC], BF) for s in range(2)]
    kout = [nc.dram_tensor(f"kout{s}", [2, 128, NDC, SC], BF) for s in range(2)]

    with tile.TileContext(nc) as tc, ExitStack() as ctx:
        consts = ctx.enter_context(tc.tile_pool(name="consts", bufs=1))
        persist = ctx.enter_context(tc.tile_pool(name="persist", bufs=1))

        ident = consts.tile([128, 128], BF, name="ident")
        make_identity(nc, ident)
        msk_sb = consts.tile([QB, KC], F32, name="msk_sb")
        nc.sync.dma_start(out=msk_sb, in_=msk_d.ap())

        # Persistent activations (partition = head dim for QT/KT, = keys for V)
        QT = persist.tile([128, NDC, SQ], BF, name="QT")   # Q^T, pre-scaled
        KT = persist.tile([128, NDC, S], BF, name="KT")    # K^T (gathered)
        V = persist.tile([128, S // 128, D], BF, name="V")  # V rows (gathered)

        # ---------------- projection phase ----------------
        with tc.tile_pool(name="proj_sb", bufs=1) as proj_sb, \
             tc.tile_pool(name="proj_ps", bufs=4, space="PSUM") as proj_ps:
            # PE warmup on the on-chip identity (no DMA dep): ramps the PE
            # p-state during the framework prologue. Result discarded.
            warm_ps = proj_ps.tile([128, SC], F32, name="warm_ps", tag="warm",
                                   bufs=1)
            for _ in range(10):
                nc.tensor.matmul(
                    warm_ps[:, 0:128], lhsT=ident, rhs=ident, start=True,
                    stop=True,
                )

            xT = proj_sb.tile([128, 4, NDC, SC], BF, name="xT")
            xth = proj_sb.tile([128, 2, NDC, SC], BF, name="xth")
            xqT = proj_sb.tile([128, SQ // SC, NDC, SC], BF, name="xqT")
            wq_sb = proj_sb.tile([128, NDC, NDC, 128], BF, name="wq_sb")
            wk_sb = proj_sb.tile([128, NDC, NDC, 128], BF, name="wk_sb")
            wv_sb = proj_sb.tile([128, NDC, D], BF, name="wv_sb")
            # single 512-col staging buffer, reused by both K stages (the
            # stage-0 DMA to kin drains well before stage 1's copies land)
            KH = proj_sb.tile([128, NDC, SC], BF, name="KH")

            # DMA priority: K-half inputs first (the exchange is on the
            # critical path), then V-half, then the Q inputs. Sync and gpsimd
            # engines issue in parallel.
            nc.sync.dma_start(out=wk_sb[:, 0], in_=wk_d.ap()[:, 0])
            nc.sync.dma_start(out=xth[:, 0], in_=xth_d.ap()[:, 0])
            for ec in range(1, NDC):
                nc.sync.dma_start(out=wk_sb[:, ec], in_=wk_d.ap()[:, ec])
            klast = nc.sync.dma_start(out=xth[:, 1], in_=xth_d.ap()[:, 1])
            i = nc.gpsimd.dma_start(out=wv_sb, in_=wv_d.ap())
            add_dep_helper(i.ins, klast.ins, reason="dma phase order")
            prev = None
            for qu in range(4):
                prev = nc.gpsimd.dma_start(
                    out=xT[:, qu], in_=xt_d.ap()[:, qu]
                )
                add_dep_helper(prev.ins, klast.ins, reason="dma phase order")
            i = nc.gpsimd.dma_start(out=wq_sb, in_=wq_d.ap())
            add_dep_helper(i.ins, prev.ins, reason="dma phase order")
            for sh in range(SQ // SC):
                i = nc.gpsimd.dma_start(out=xqT[:, sh], in_=xqt_d.ap()[:, sh])
                add_dep_helper(i.ins, prev.ins, reason="dma phase order")

            def k_half(sh):
                # K^T[e, own-half tokens sh*512..] = sum_d Wk[d,e] xh^T[d, s]
                for ec in range(NDC):
                    pp = proj_ps.tile([128, SC], F32, name="pp")
                    for dc in range(NDC):
                        nc.tensor.matmul(
                            pp,
                            lhsT=wk_sb[:, ec, dc, :],
                            rhs=xth[:, sh, dc, :],
                            start=(dc == 0),
                            stop=(dc == NDC - 1),
                        )
                    nc.scalar.copy(KH[:, ec, :], pp)
                # stage + exchange. Tokens are interleaved between the pair by
                # 256-chunks (rank r owns global chunks {2*lc + r}), so this
                # stage's AllGather delivers a CONTIGUOUS early key range
                # [s*1024, (s+1)*1024) — matching the causal need order.
                d1 = nc.sync.dma_start(out=kin[sh].ap(), in_=KH)
                cc = nc.gpsimd.collective_compute(
                    "AllGather", mybir.AluOpType.bypass,
                    replica_groups=GROUPS,
                    ins=[kin[sh].ap().opt()],
                    outs=[kout[sh].ap().opt()],
                )
                add_dep_helper(cc.ins, d1.ins, reason="cc after staging dma")
                for r in range(2):
                    for m in range(2):
                        g = 2 * (2 * sh + m) + r   # global 256-chunk index
                        d2 = nc.sync.dma_start(
                            out=KT[:, :, g * KC:(g + 1) * KC],
                            in_=kout[sh].ap()[r][:, :, m * KC:(m + 1) * KC],
                        )
                        add_dep_helper(d2.ins, cc.ins, reason="read gathered")

            # K halves + exchanges first (max collective lead time),
            # then the full (duplicated) V projection, then Q
            k_half(0)
            k_half(1)

            # V[s, e] = sum_d x^T[d, s] * Wv[d, e]
            for kb in range(S // 128):
                for eh in range(D // SC):
                    pp = proj_ps.tile([128, SC], F32, name="pp")
                    for dc in range(NDC):
                        nc.tensor.matmul(
                            pp,
                            lhsT=xT[:, kb // 4, dc,
                                    (kb % 4) * 128:(kb % 4 + 1) * 128],
                            rhs=wv_sb[:, dc, eh * SC:(eh + 1) * SC],
                            start=(dc == 0),
                            stop=(dc == NDC - 1),
                        )
                    nc.scalar.copy(V[:, kb, eh * SC:(eh + 1) * SC], pp)

            # Q^T[e, s] = sum_d Wq[d, e] * xq^T[d, s]   (scale folded in)
            for sh in range(SQ // SC):
                for ec in range(NDC):
                    pp = proj_ps.tile([128, SC], F32, name="pp")
                    for dc in range(NDC):
                        nc.tensor.matmul(
                            pp,
                            lhsT=wq_sb[:, ec, dc, :],
                            rhs=xqT[:, sh, dc, :],
                            start=(dc == 0),
                            stop=(dc == NDC - 1),
                        )
                    nc.scalar.mul(QT[:, ec, sh * SC:(sh + 1) * SC], pp, SCALE)

        # ---------------- attention phase ----------------
        with tc.tile_pool(name="att_sb", bufs=2) as att_sb, \
             tc.tile_pool(name="pt_sb_pool", bufs=20) as pt_pool, \
             tc.tile_pool(name="stat_sb", bufs=4) as stat_sb, \
             tc.tile_pool(name="att_ps", bufs=1, space="PSUM") as att_ps:
            for j in reversed(range(NQB)):
                ext = (j + 1) * KC            # causal key extent for block j
                nch = (ext + SC - 1) // SC    # score chunks (512 wide, last may be 256)
                qsl = slice(j * 128, (j + 1) * 128)

                # Scores are ~N(0,1) after the folded 1/sqrt(D) scaling, so
                # exp() without max-subtraction is numerically safe; dropping
                # the global row-max removes the per-block barrier and lets
                # scores -> exp -> transpose -> AV pipeline per 512-chunk.
                P = att_sb.tile([128, NQB * KC], BF, name="P", tag="P")
                rsum = stat_sb.tile([128, nch], F32, name="rsum", tag="rsum")
                pts = []

                # software pipeline: scores(c+1) is issued on the PE before
                # the transposes of chunk c, so the PE crunches the next
                # chunk's scores while the scalar engine runs exp(c).
                def do_scores(c):
                    w = min(SC, ext - c * SC)
                    ps_c = att_ps.tile([128, SC], F32, name="ps_sc",
                                       tag="ps_sc", bufs=4)
                    for ec in range(NDC):
                        nc.tensor.matmul(
                            ps_c[:, 0:w],
                            lhsT=QT[:, ec, qsl],
                            rhs=KT[:, ec, c * SC:c * SC + w],
                            start=(ec == 0),
                            stop=(ec == NDC - 1),
                        )
                    if c == nch - 1:
                        # causal mask on the last KC columns
                        nc.vector.tensor_add(
                            out=ps_c[:, w - KC:w],
                            in0=ps_c[:, w - KC:w],
                            in1=msk_sb,
                        )
                    return ps_c, w

                def do_exp_transpose(c, ps_c, w):
                    nc.scalar.activation(
                        P[:, c * SC:c * SC + w],
                        ps_c[:, 0:w],
                        mybir.ActivationFunctionType.Exp,
                        bias=0.0,
                        scale=1.0,
                        accum_out=rsum[:, c:c + 1],
                    )
                    for kb in range(w // 128):
                        pt_ps = att_ps.tile(
                            [128, 128], BF, name="pt_ps", tag="pt_ps", bufs=2
                        )
                        nc.tensor.transpose(
                            pt_ps,
                            P[:, c * SC + kb * 128:c * SC + (kb + 1) * 128],
                            ident,
                        )
                        pt = pt_pool.tile([128, 128], BF, name="pt", tag="pt")
                        nc.vector.tensor_copy(pt, pt_ps)
                        pts.append(pt)

                pend = do_scores(0)
                for c in range(nch):
                    nxt = do_scores(c + 1) if c + 1 < nch else None
                    do_exp_transpose(c, *pend)
                    pend = nxt

                tsum = stat_sb.tile([128, 1], F32, name="tsum", tag="tsum")
                nc.vector.reduce_sum(tsum, rsum, axis=mybir.AxisListType.X)
                rinv = stat_sb.tile([128, 1], F32, name="rinv", tag="rinv")
                nc.vector.reciprocal(rinv, tsum)

                # out[q, e] = sum_k P^T[k, q]^T V[k, e]
                ps_o = []
                for eh in range(D // SC):
                    # the final block borrows the bank-padded transpose-staging
                    # slots so it never waits on the previous block's normalize
                    avtag = "pt_ps" if j == 0 else "ps_av"
                    ps_av = att_ps.tile([128, SC], F32, name="ps_av",
                                        tag=avtag, bufs=2)
                    for kb in range(ext // 128):
                        nc.tensor.matmul(
                            ps_av,
                            lhsT=pts[kb],
                            rhs=V[:, kb, eh * SC:(eh + 1) * SC],
                            start=(kb == 0),
                            stop=(kb == ext // 128 - 1),
                        )
                    ps_o.append(ps_av)

                # normalize + store per half so the first DMA overlaps the
                # second normalize (matters for the last block's tail)
                ob = att_sb.tile([128, D], F32, name="ob", tag="ob")
                for eh in range(D // SC):
                    nc.scalar.mul(ob[:, eh * SC:(eh + 1) * SC], ps_o[eh], rinv)
                    nc.sync.dma_start(
                        out=out_d.ap()[j][:, eh * SC:(eh + 1) * SC],
                        in_=ob[:, eh * SC:(eh + 1) * SC],
                    )

    nc.compile()
    return nc


_PROGRAM = None


def _get_program():
    global _PROGRAM
    if _PROGRAM is None:
        _PROGRAM = build_program()
    return _PROGRAM


def _pack_w(w):
    # [D, D] -> [128, NDC, D]: partition p, chunk dc holds row dc*128+p
    bf = ml_dtypes.bfloat16
    return np.ascontiguousarray(
        w.astype(bf).reshape(NDC, 128, D).transpose(1, 0, 2)
    )


def _pack_w_ec(w):
    # ec-major: [128, ec, dc, 128] so each ec-chunk is one small priority DMA
    return np.ascontiguousarray(
        _pack_w(w).reshape(128, NDC, NDC, 128).transpose(0, 2, 1, 3)
    )


def _pack_xt(xr):
    # [rows, D] -> x^T packed [128, NDC, rows]
    bf = ml_dtypes.bfloat16
    return np.ascontiguousarray(
        xr.astype(bf).T.reshape(NDC, 128, xr.shape[0]).transpose(1, 0, 2)
    )


def make_in_maps(x, Wq, Wk, Wv):
    wqb = _pack_w_ec(Wq)
    wkb = _pack_w_ec(Wk)
    wvb = _pack_w(Wv)
    r = np.arange(QB)[:, None]
    cc = np.arange(KC)[None, :]
    in_maps = []
    for c in range(NCORES):
        b, t = c // 2, c % 2
        xb = x[b]
        xqb = xb.reshape(S // QB, QB, D)[t::2].reshape(SQ, D)
        xqtb = np.ascontiguousarray(
            _pack_xt(xqb).reshape(128, NDC, SQ // SC, SC).transpose(0, 2, 1, 3)
        )
        # own tokens = every other 256-chunk (rank r owns global chunks
        # {2*lc + r}), packed [128, shh, dc, 512]
        xh_rows = xb.reshape(S // KC, KC, D)[t::2].reshape(SH, D)
        xhb = np.ascontiguousarray(
            _pack_xt(xh_rows)
            .reshape(128, NDC, 2, SC).transpose(0, 2, 1, 3)
        )
        mask = np.where(cc <= t * QB + r, 0.0, MASK_VAL).astype(np.float32)
        xtb = np.ascontiguousarray(
            _pack_xt(xb).reshape(128, NDC, 4, SC).transpose(0, 2, 1, 3)
        )
        in_maps.append(
            {"xt": xtb, "xth": xhb, "xqt": xqtb,
             "wq": wqb, "wk": wkb, "wv": wvb, "msk": mask}
        )
    return in_maps


def assemble_output(results):
    out = np.empty((B, S, D), dtype=np.float32)
    ov = out.reshape(B, S // QB, QB, D)
    for c in range(NCORES):
        b, t = c // 2, c % 2
        ov[b, t::2] = results[c]["out"]
    return out


def kernel(x, Wq, Wk, Wv):
    x = np.asarray(x)
    nc = _get_program()
    in_maps = make_in_maps(x, np.asarray(Wq), np.asarray(Wk), np.asarray(Wv))
    res = run_bass_kernel_spmd(nc, in_maps, list(range(NCORES))).results
    return assemble_output(res)
